# revision 7
# baseline (speedup 1.0000x reference)
"""Tri-quadratic (order-3) tensor-product B-spline evaluation at 2M points,
computed on 8 NeuronCores.

Pipeline
--------
Host (cheap numpy): clip/scale uvw, split each coordinate into a 6-bit
interval index i and 10-bit quantized fraction, pack as uint16; compute a
15-bit overlapping-tile id per point (int16) laid out in the gather wrap
order the GPSIMD DMA-gather expects. Device (Bass, SPMD over 8 cores):
per 8192-point batch, one 512-byte dma_gather descriptor per point pulls
the point's 4x4x4x3 coefficient neighborhood (an overlapping-tile fp16
table baked into the NEFF as a Const tensor, so it never crosses the
host link), the quadratic basis is evaluated in f32, folded with the
intra-tile offset into per-axis 4-tap weight vectors, and the 192-tap
contraction runs as three multiply + masked-scan (segmented-sum) stages
in fp16 with f32 scan state. Host unscrambles the fp16 result to f32.

The coefficient table depends only on `coeff`; the compiled program and
jitted dispatcher are cached across calls keyed on the coeff bytes.
If anything about the device path fails, a numpy fallback (bit-identical
to the reference up to f32 rounding) computes the result instead.
"""

import numpy as np

F32 = np.float32
NP_TOTAL = 2_000_000
N_CORES = 8
P = 128            # SBUF partitions
FPTS = 64          # points per partition per batch
BATCH = P * FPTS   # 8192 points per batch
NB = 31            # batches per core; covers 253952 >= 250000 points
CORE_PTS = NB * BATCH
NGRID = 64
NSEG = 62
NTILE = 31         # overlapping 4^3 tiles with stride 2 per axis
TROWS = NTILE ** 3  # 29791 table rows
TREC = 256         # fp16 elems per table row (192 payload + 64 pad) = 512B


# ---------------------------------------------------------------------------
# Host-side packing
# ---------------------------------------------------------------------------

def _quantize(uvw):
    """uvw [3, N] f32 -> q uint16 [3, N] (i<<10|fq), tid int16 [N]."""
    X = np.clip(uvw, F32(1e-14), F32(1.0) - F32(1e-14)).astype(F32)
    t = (X * F32(62.0)).astype(F32)
    i = np.ceil(t).astype(F32) - F32(1.0)
    np.clip(i, 0.0, 61.0, out=i)
    f = (t - i).astype(F32)
    fq = np.rint(f * F32(1023.0)).astype(np.uint16)
    ii = i.astype(np.uint16)
    q = (ii << 10) | fq
    iq = ii >> 1
    tid = (iq[0].astype(np.int32) * 961 + iq[1].astype(np.int32) * 31
           + iq[2].astype(np.int32)).astype(np.int16)
    return q, tid


def _build_table(coeff):
    """fp16 table [TROWS, TREC]; row (qu,qv,qw) = coeff[:, 2qu:2qu+4, ...]
    laid out [c, x, y, z] in the first 192 elems."""
    sw = np.lib.stride_tricks.sliding_window_view(coeff, (4, 4, 4), axis=(1, 2, 3))
    sw = sw[:, ::2, ::2, ::2]                        # [3,31,31,31,4,4,4]
    tbl = np.zeros((TROWS, TREC), np.float16)
    tbl[:, :192] = np.moveaxis(sw, 0, 3).reshape(TROWS, 192)
    return tbl


def _pack_inputs(uvw):
    """-> Q [8*NB, P, 3, FPTS] u16, A [8*NB, 16, BATCH//16] i16"""
    q, tid = _quantize(uvw)
    Nc = NP_TOTAL // N_CORES
    qp = np.zeros((3, N_CORES, CORE_PTS), np.uint16)
    tp = np.zeros((N_CORES, CORE_PTS), np.int16)
    qp[:, :, :Nc] = q.reshape(3, N_CORES, Nc)
    tp[:, :Nc] = tid.reshape(N_CORES, Nc)
    # Q: per core [NB, P, 3, F]
    Q = (qp.reshape(3, N_CORES, NB, P, FPTS)
         .transpose(1, 2, 3, 0, 4)
         .reshape(N_CORES * NB, P, 3, FPTS))
    Q = np.ascontiguousarray(Q)
    # A: gather order i = g*128 + p maps to point n = b*BATCH + p*F + g;
    # wrapped [16, i//16]: i = s*16 + qq
    A = (tp.reshape(N_CORES, NB, P, FPTS)
         .transpose(0, 1, 3, 2)                       # [8, NB, g, p]
         .reshape(N_CORES * NB, BATCH // 16, 16)      # [.., s, qq]
         .transpose(0, 2, 1))                          # [.., 16, s]
    A = np.ascontiguousarray(A)
    return Q, A


def _unpack_output(raw):
    """raw fp16 [8*NB, P, 3*FPTS] -> xyz f32 [3, NP_TOTAL]."""
    Nc = NP_TOTAL // N_CORES
    r = raw.reshape(N_CORES, CORE_PTS, 3)
    out = np.empty((3, NP_TOTAL), np.float32)
    for c in range(N_CORES):
        out[:, c * Nc:(c + 1) * Nc] = r[c, :Nc, :].T.astype(np.float32)
    return out


# ---------------------------------------------------------------------------
# Device program
# ---------------------------------------------------------------------------

def _build_program(table):
    import concourse.bass as bass
    import concourse.tile as tile
    from concourse import bacc, mybir
    from contextlib import ExitStack

    dt = mybir.dt
    op = mybir.AluOpType
    F = FPTS

    nc = bacc.Bacc("TRN2", target_bir_lowering=False, debug=False)
    q_d = nc.dram_tensor("qpk", [NB, P, 3, F], dt.uint16, kind="ExternalInput")
    a_d = nc.dram_tensor("tidx", [NB, 16, BATCH // 16], dt.int16, kind="ExternalInput")
    o_d = nc.dram_tensor("xyzo", [NB, P, 3 * F], dt.float16, kind="ExternalOutput")
    t_d = nc.inline_tensor(table, name="tbl")

    with nc.allow_low_precision(reason="fp16 partials; scan state is f32"):
        with tile.TileContext(nc) as tc:
            with ExitStack() as ctx:
                cpool = ctx.enter_context(tc.tile_pool(name="c", bufs=1))
                pool = ctx.enter_context(tc.tile_pool(name="p", bufs=2))

                # constant scan mask: repeating [0,1,1,1] fp16
                mask = cpool.tile([P, F * 192], dt.float16, tag="mask")
                nc.vector.memset(mask[:], 1.0)
                nc.vector.memset(
                    mask[:].rearrange("p (s z) -> p s z", z=4)[:, :, 0], 0.0)

                for b in range(NB):
                    qt = pool.tile([P, 3, F], dt.uint16, tag="q")
                    nc.sync.dma_start(qt[:], q_d.ap()[b])
                    idx = pool.tile([P, BATCH // 16], dt.int16, tag="idx")
                    nc.sync.dma_start(idx[0:16, :], a_d.ap()[b])
                    for j in range(1, 8):
                        nc.sync.dma_start(idx[16 * j:16 * (j + 1), :], idx[0:16, :])

                    win = pool.tile([P, F, TREC], dt.float16, tag="win")
                    nc.gpsimd.dma_gather(
                        win[:], t_d.ap(), idx[:], BATCH, BATCH, TREC)

                    # ---- unpack q -> i, f, du (f32), all 3 dims at once
                    i16 = pool.tile([P, 3, F], dt.uint16, tag="i16")
                    f16i = pool.tile([P, 3, F], dt.uint16, tag="f16i")
                    d16 = pool.tile([P, 3, F], dt.uint16, tag="d16")
                    nc.vector.tensor_scalar(i16[:], qt[:], 10, None, op.logical_shift_right)
                    nc.vector.tensor_scalar(f16i[:], qt[:], 1023, None, op.bitwise_and)
                    nc.vector.tensor_scalar(d16[:], qt[:], 10, 1, op.logical_shift_right, op.bitwise_and)
                    iuf = pool.tile([P, 3, F], dt.float32, tag="iuf")
                    ff = pool.tile([P, 3, F], dt.float32, tag="ff")
                    duf = pool.tile([P, 3, F], dt.float32, tag="duf")
                    nc.vector.tensor_copy(iuf[:], i16[:])
                    nc.vector.tensor_copy(ff[:], f16i[:])
                    nc.vector.tensor_copy(duf[:], d16[:])
                    nc.vector.tensor_scalar(ff[:], ff[:], float(1.0 / 1023.0), None, op.mult)

                    # ---- basis N0/N1/N2 -> nub [P, 3, F, 5] rows 1..3
                    rd0 = pool.tile([P, 3, F], dt.float32, tag="rd0")
                    rd2 = pool.tile([P, 3, F], dt.float32, tag="rd2")
                    nc.vector.tensor_scalar(rd0[:], iuf[:], 0.0, 0.5, op.is_equal, op.mult)
                    nc.vector.tensor_scalar(rd0[:], rd0[:], 0.5, None, op.add)
                    nc.vector.tensor_scalar(rd2[:], iuf[:], 61.0, 0.5, op.is_equal, op.mult)
                    nc.vector.tensor_scalar(rd2[:], rd2[:], 0.5, None, op.add)
                    omf = pool.tile([P, 3, F], dt.float32, tag="omf")
                    nc.vector.tensor_scalar(omf[:], ff[:], -1.0, 1.0, op.mult, op.add)
                    sq = pool.tile([P, 3, F], dt.float32, tag="sq")
                    nub = pool.tile([P, 3, F, 5], dt.float32, tag="nub")
                    nc.vector.memset(nub[:], 0.0)
                    nc.vector.tensor_tensor(sq[:], omf[:], omf[:], op.mult)
                    nc.vector.tensor_tensor(nub[:, :, :, 1], sq[:], rd0[:], op.mult)
                    nc.vector.tensor_tensor(sq[:], ff[:], ff[:], op.mult)
                    nc.vector.tensor_tensor(nub[:, :, :, 3], sq[:], rd2[:], op.mult)
                    nc.vector.tensor_tensor(sq[:], nub[:, :, :, 1], nub[:, :, :, 3], op.add)
                    nc.vector.tensor_scalar(nub[:, :, :, 2], sq[:], -1.0, 1.0, op.mult, op.add)

                    # ---- extended 4-tap weights nup fp16 [P, 3, F, 4]
                    diff = pool.tile([P, 3, F, 4], dt.float32, tag="diff")
                    nup = pool.tile([P, 3, F, 4], dt.float16, tag="nup")
                    nc.vector.tensor_tensor(diff[:], nub[:, :, :, 0:4], nub[:, :, :, 1:5], op.subtract)
                    nc.vector.tensor_tensor(diff[:], diff[:], duf[:].broadcast_to([P, 3, F, 4]), op.mult)
                    nc.vector.tensor_tensor(nup[:], diff[:], nub[:, :, :, 1:5], op.add)

                    # ---- contraction: [c,x,y,z] win * nw -> scan z -> * nv
                    #      -> scan y -> * nu -> scan x
                    prodz = pool.tile([P, F * 192], dt.float16, tag="prodz")
                    nc.vector.tensor_tensor(
                        prodz[:].rearrange("p (f s z) -> p f s z", s=48, z=4),
                        win[:].rearrange("p f (s z) -> p f s z", z=4)[:, :, 0:48, :],
                        nup[:, 2, :, :].unsqueeze(2).broadcast_to([P, F, 48, 4]),
                        op.mult)
                    nc.vector.tensor_tensor_scan(
                        prodz[:], mask[:], prodz[:], 0.0, op.mult, op.add)
                    prody = pool.tile([P, F * 48], dt.float16, tag="prody")
                    prody_v = prody[:].rearrange("p (f c x y) -> p f c x y",
                                                 c=3, x=4, y=4)
                    zscan_v = prodz[:].rearrange("p (f c x y z) -> p f c x y z",
                                                 c=3, x=4, y=4, z=4)
                    for cc in range(3):
                        nc.vector.tensor_tensor(
                            prody_v[:, :, cc, :, :],
                            zscan_v[:, :, cc, :, :, 3],
                            nup[:, 1, :, :].unsqueeze(2).broadcast_to([P, F, 4, 4]),
                            op.mult)
                    nc.vector.tensor_tensor_scan(
                        prody[:], mask[:, 0:F * 48], prody[:], 0.0, op.mult, op.add)
                    prodx = pool.tile([P, F * 12], dt.float16, tag="prodx")
                    nc.vector.tensor_tensor(
                        prodx[:].rearrange("p (f c x) -> p f c x", c=3, x=4),
                        prody[:].rearrange("p (f c x y) -> p f c x y",
                                           c=3, x=4, y=4)[:, :, :, :, 3],
                        nup[:, 0, :, :].unsqueeze(2).broadcast_to([P, F, 3, 4]),
                        op.mult)
                    nc.vector.tensor_tensor_scan(
                        prodx[:], mask[:, 0:F * 12], prodx[:], 0.0, op.mult, op.add)
                    outt = pool.tile([P, F * 3], dt.float16, tag="outt")
                    nc.vector.tensor_copy(
                        outt[:],
                        prodx[:].rearrange("p (f c x) -> p f c x", c=3, x=4)[:, :, :, 3])
                    nc.sync.dma_start(o_d.ap()[b], outt[:])

    nc.compile()
    return nc


# ---------------------------------------------------------------------------
# Dispatch (cached jitted shard_map over 8 cores)
# ---------------------------------------------------------------------------

_STATE = {"key": None, "fn": None, "fail": False}


def _make_dispatch(nc):
    import jax
    import jax.numpy as jnp
    from jax.sharding import Mesh, PartitionSpec
    from jax.experimental.shard_map import shard_map
    from concourse import mybir
    from concourse.bass2jax import (_bass_exec_p, partition_id_tensor,
                                    install_neuronx_cc_hook)

    install_neuronx_cc_hook()

    in_names, out_names, out_avals = [], [], []
    partition_name = (nc.partition_id_tensor.name
                      if nc.partition_id_tensor is not None else None)
    for alloc in nc.m.functions[0].allocations:
        if not isinstance(alloc, mybir.MemoryLocationSet):
            continue
        name = alloc.memorylocations[0].name
        if alloc.kind == "ExternalInput":
            if name != partition_name:
                in_names.append(name)
        elif alloc.kind == "ExternalOutput":
            out_names.append(name)
            out_avals.append(jax.core.ShapedArray(
                tuple(alloc.tensor_shape), mybir.dt.np(alloc.dtype)))
    n_params = len(in_names)
    all_names = list(in_names) + list(out_names)
    if partition_name is not None:
        all_names.append(partition_name)

    def _body(*args):
        operands = list(args)
        if partition_name is not None:
            operands.append(partition_id_tensor())
        outs = _bass_exec_p.bind(
            *operands,
            out_avals=tuple(out_avals),
            in_names=tuple(all_names),
            out_names=tuple(out_names),
            lowering_input_output_aliases=(),
            sim_require_finite=False,
            sim_require_nnan=False,
            nc=nc,
        )
        return tuple(outs)

    n_out = len(out_names)
    devices = jax.devices()[:N_CORES]
    mesh = Mesh(np.asarray(devices), ("core",))
    fn = jax.jit(shard_map(
        _body, mesh=mesh,
        in_specs=(PartitionSpec("core"),) * (n_params + n_out),
        out_specs=(PartitionSpec("core"),) * n_out,
        check_rep=False),
        donate_argnums=tuple(range(n_params, n_params + n_out)))
    out_global = [(tuple([N_CORES * av.shape[0]] + list(av.shape[1:])), av.dtype)
                  for av in out_avals]
    return fn, in_names, out_names, out_global


def _get_dispatch(coeff):
    key = hash(coeff.tobytes())
    if _STATE["key"] == key:
        return _STATE["fn"]
    table = _build_table(coeff)
    nc = _build_program(table)
    fn = _make_dispatch(nc)
    _STATE["key"] = key
    _STATE["fn"] = fn
    _STATE["nc"] = nc
    _STATE["outbufs"] = None
    return fn


# ---------------------------------------------------------------------------
# Host fallback (matches reference to ~1e-5)
# ---------------------------------------------------------------------------

def _basis_f32(X):
    X = np.maximum(X, F32(1e-14)).astype(F32)
    t = (X * F32(62.0)).astype(F32)
    C = F32(2 ** 23)
    r = ((t + C) - C).astype(F32)
    g = (t > r).astype(F32)
    i = (r + g - F32(1.0)).astype(F32)
    np.clip(i, F32(0.0), F32(61.0), out=i)
    f = (t - i).astype(F32)
    omf = (F32(1.0) - f).astype(F32)
    eq0 = (i == F32(0.0)).astype(F32)
    eq61 = (i == F32(61.0)).astype(F32)
    rD31 = (eq0 * F32(0.5) + F32(0.5)).astype(F32)
    rD42 = (eq61 * F32(0.5) + F32(0.5)).astype(F32)
    N0 = (omf * omf * rD31).astype(F32)
    N2 = (f * f * rD42).astype(F32)
    N1 = ((F32(1.0) - N0) - N2).astype(F32)
    return i.astype(np.int64), N0, N1, N2


def _spline_eval(uvw, coeff, chunk=262144):
    iu, NU0, NU1, NU2 = _basis_f32(uvw[0])
    iv, NV0, NV1, NV2 = _basis_f32(uvw[1])
    iw, NW0, NW1, NW2 = _basis_f32(uvw[2])
    NU = (NU0, NU1, NU2)
    NV = (NV0, NV1, NV2)
    NW = (NW0, NW1, NW2)
    cf = np.ascontiguousarray(coeff.reshape(3, -1))
    V = np.lib.stride_tricks.sliding_window_view(cf, 3, axis=1)
    base = (iu.astype(np.int32) * np.int32(NGRID * NGRID)
            + iv.astype(np.int32) * np.int32(NGRID) + iw.astype(np.int32))
    N = uvw.shape[1]
    out = np.empty((3, N), dtype=F32)
    for s in range(0, N, chunk):
        e = min(s + chunk, N)
        b = base[s:e]
        acc = np.zeros((3, e - s), dtype=F32)
        for ii in range(3):
            for jj in range(3):
                idx = b + np.int32(ii * NGRID * NGRID + jj * NGRID)
                G = V[:, idx, :]
                wuv = NU[ii][s:e] * NV[jj][s:e]
                w0 = wuv * NW[0][s:e]
                w1 = wuv * NW[1][s:e]
                w2 = wuv * NW[2][s:e]
                acc += G[:, :, 0] * w0 + G[:, :, 1] * w1 + G[:, :, 2] * w2
        out[:, s:e] = acc
    return out


# ---------------------------------------------------------------------------
# Entry point
# ---------------------------------------------------------------------------

def kernel(uvw, knotx, knoty, knotz, coeff, order):
    uvw = np.asarray(uvw, dtype=np.float32)
    coeff = np.asarray(coeff, dtype=np.float32)
    if not _STATE["fail"]:
        try:
            fn, in_names, out_names, out_global = _get_dispatch(coeff)
            Q, A = _pack_inputs(uvw)
            feed = {"qpk": Q, "tidx": A}
            outbufs = _STATE.get("outbufs")
            if outbufs is None:
                outbufs = [np.zeros(s, d) for s, d in out_global]
            outs = fn(*[feed[n] for n in in_names], *outbufs)
            # recycle device-resident outputs as next call's donated buffers
            _STATE["outbufs"] = list(outs)
            raw = np.asarray(outs[out_names.index("xyzo")])
            return _unpack_output(raw)
        except Exception:
            import traceback
            traceback.print_exc()
            _STATE["fail"] = True
            _STATE["outbufs"] = None
    return _spline_eval(uvw, coeff).astype(np.float32)
